# revision 5
# baseline (speedup 1.0000x reference)
"""GC-LSTM (ChebConv K=1 + peephole LSTM + mean-pool) on 8 TRN2 NeuronCores.

Strategy: data-parallel over the node axis N = B*A = 1024 -> 128 nodes/core.
Gate weights are replicated; the T=128 recurrence runs fully on-chip per core
with no cross-device traffic. The final mean-pool over agents + concat is done
on host (tiny).

Per-core layout ("option A"):
  - pre_t = [x_t;1] @ [Wx;bias] + h_{t-1} @ Wh computed as PSUM [128 nodes, 1536]
    with lhsT = transposed activations (xT 33x128 / hT 128x128 chunks) and
    rhs = weight column slices (N=512 -> float32r runs at 1 cycle/row).
  - gate order in the 1536 columns is permuted to [f, i, g, o] (host-side
    weight permutation) so the f-gate (head of the critical path) is ready first.
  - elementwise gates in [nodes=128 partitions, HID=384 free]; peephole weight
    vectors are pre-broadcast to [128, 384] tiles on host.
  - h is transposed back to [HID, nodes] chunks via PE transpose + copy for the
    next step's matmul.
"""

import numpy as np

import concourse.bass as bass
from concourse import bacc
import concourse.mybir as mybir
import concourse.tile as tile
from concourse.bass_utils import run_bass_kernel_spmd
from concourse.masks import make_identity

AF = mybir.ActivationFunctionType
F32 = mybir.dt.float32
F32R = mybir.dt.float32r

B, T, A, F_IN, HID = 64, 128, 16, 32, 384
H4 = 4 * HID
N_CORES = 8
NPC = (B * A) // N_CORES  # nodes per core = 128
KC = HID // 128           # h contraction chunks = 3
XR = F_IN + 1             # x rows + ones row for bias = 33
NXT = 4                   # xs DMA chunks

_cache = {}


def _build_nc():
    nc = bacc.Bacc("TRN2", target_bir_lowering=False, debug=False)

    d_xs = nc.dram_tensor("xs", [XR, T * NPC], F32R, kind="ExternalInput")
    d_wxa = nc.dram_tensor("wxa", [XR, H4], F32R, kind="ExternalInput")
    d_wh = nc.dram_tensor("wh", [HID, H4], F32R, kind="ExternalInput")
    d_wci = nc.dram_tensor("wci", [NPC, HID], F32, kind="ExternalInput")
    d_wcf = nc.dram_tensor("wcf", [NPC, HID], F32, kind="ExternalInput")
    d_wco = nc.dram_tensor("wco", [NPC, HID], F32, kind="ExternalInput")
    d_out = nc.dram_tensor("out", [NPC, HID], F32, kind="ExternalOutput")

    with tile.TileContext(nc) as tc:
        with (
            tc.tile_pool(name="consts", bufs=1) as cp,
            tc.tile_pool(name="work", bufs=2) as wp,
            tc.tile_pool(name="psum", bufs=2, space="PSUM") as pp,
        ):
            # ---------------- constants (DMA'd once) ----------------
            wxa = cp.tile([XR, H4], F32R, tag="wxa", name="wxa")
            nc.sync.dma_start(wxa, d_wxa.ap())
            wh = cp.tile([128, KC, H4], F32R, tag="wh", name="wh")
            for k in range(KC):
                nc.sync.dma_start(wh[:, k, :], d_wh.ap()[k * 128:(k + 1) * 128, :])
            wci = cp.tile([NPC, HID], F32, tag="wci", name="wci")
            nc.sync.dma_start(wci, d_wci.ap())
            wcf = cp.tile([NPC, HID], F32, tag="wcf", name="wcf")
            nc.sync.dma_start(wcf, d_wcf.ap())
            wco = cp.tile([NPC, HID], F32, tag="wco", name="wco")
            nc.sync.dma_start(wco, d_wco.ap())
            xs_t = []
            xcols = T * NPC // NXT
            for q in range(NXT):
                xt = cp.tile([XR, xcols], F32R, tag=f"xs{q}", name=f"xs{q}")
                nc.sync.dma_start(xt, d_xs.ap()[:, q * xcols:(q + 1) * xcols])
                xs_t.append(xt)
            ident = cp.tile([128, 128], F32, tag="ident", name="ident")
            make_identity(nc, ident)

            # ---------------- state init ----------------
            c_prev = wp.tile([NPC, HID], F32, tag="c", bufs=2, name="c_init")
            nc.vector.memset(c_prev, 0.0)
            hT = []
            for k in range(KC):
                t0 = wp.tile([128, NPC], F32R, tag=f"hT{k}", bufs=2, name=f"hT_init{k}")
                nc.gpsimd.memset(t0.bitcast(F32), 0.0)
                hT.append(t0)

            # ---------------- time loop (fully unrolled) ----------------
            h = None
            for t in range(T):
                q, r = divmod(t, T // NXT)
                xT = xs_t[q][:, r * NPC:(r + 1) * NPC]

                pre = pp.tile([NPC, H4], F32, tag="pre", bufs=2, name=f"pre{t}")
                for j in range(3):
                    js = slice(512 * j, 512 * (j + 1))
                    nc.tensor.matmul(
                        pre[:, js], xT, wxa[:, js],
                        start=True, stop=False,
                    )
                for k in range(KC):
                    for j in range(3):
                        js = slice(512 * j, 512 * (j + 1))
                        nc.tensor.matmul(
                            pre[:, js], hT[k], wh[:, k, js],
                            start=False, stop=(k == KC - 1),
                        )

                # gates: columns are [f, i, g, o] after host permutation
                tf_ = wp.tile([NPC, HID], F32, tag="tf", bufs=2, name=f"tf{t}")
                nc.vector.tensor_mul(tf_, c_prev, wcf)
                ti_ = wp.tile([NPC, HID], F32, tag="ti", bufs=2, name=f"ti{t}")
                nc.vector.tensor_mul(ti_, c_prev, wci)
                sif = wp.tile([NPC, 2 * HID], F32, tag="sif", bufs=2, name=f"sif{t}")
                nc.vector.tensor_add(sif[:, 0:HID], tf_, pre[:, 0:HID])
                nc.vector.tensor_add(sif[:, HID:2 * HID], ti_, pre[:, HID:2 * HID])
                fi = wp.tile([NPC, 2 * HID], F32, tag="fi", bufs=2, name=f"fi{t}")
                nc.scalar.activation(fi, sif, AF.Sigmoid)
                g = wp.tile([NPC, HID], F32, tag="g", bufs=2, name=f"g{t}")
                nc.scalar.activation(g, pre[:, 2 * HID:3 * HID], AF.Tanh)

                t1 = wp.tile([NPC, HID], F32, tag="t1", bufs=2, name=f"t1_{t}")
                nc.vector.tensor_mul(t1, fi[:, 0:HID], c_prev)
                t2 = wp.tile([NPC, HID], F32, tag="t2", bufs=2, name=f"t2_{t}")
                nc.vector.tensor_mul(t2, fi[:, HID:2 * HID], g)
                c_new = wp.tile([NPC, HID], F32, tag="c", bufs=2, name=f"c{t}")
                nc.vector.tensor_add(c_new, t1, t2)

                to_ = wp.tile([NPC, HID], F32, tag="to", bufs=2, name=f"to{t}")
                nc.vector.tensor_mul(to_, c_new, wco)
                so_ = wp.tile([NPC, HID], F32, tag="so", bufs=2, name=f"so{t}")
                nc.vector.tensor_add(so_, to_, pre[:, 3 * HID:4 * HID])
                o_ = wp.tile([NPC, HID], F32, tag="o", bufs=2, name=f"o{t}")
                nc.scalar.activation(o_, so_, AF.Sigmoid)
                th = wp.tile([NPC, HID], F32, tag="th", bufs=2, name=f"th{t}")
                nc.scalar.activation(th, c_new, AF.Tanh)
                h = wp.tile([NPC, HID], F32, tag="h", bufs=2, name=f"h{t}")
                nc.vector.tensor_mul(h, o_, th)

                if t != T - 1:
                    for k in range(KC):
                        pT = pp.tile([128, NPC], F32, tag="pT", bufs=2,
                                     name=f"pT{t}_{k}")
                        nc.tensor.transpose(pT, h[:, 128 * k:128 * (k + 1)], ident)
                        hTn = wp.tile([128, NPC], F32R, tag=f"hT{k}", bufs=2,
                                      name=f"hT{t}_{k}")
                        nc.scalar.copy(hTn, pT)
                        hT[k] = hTn
                c_prev = c_new

            nc.sync.dma_start(d_out.ap(), h)

    nc.compile()
    return nc


def _get_nc():
    if "nc" not in _cache:
        _cache["nc"] = _build_nc()
    return _cache["nc"]


def _prep_in_maps(agent_obs, Wx, Wh, b, b_conv, w_ci, w_cf, w_co):
    # permute gate blocks [i, f, c, o] -> [f, i, g, o]
    perm = np.concatenate([
        np.arange(HID, 2 * HID), np.arange(0, HID),
        np.arange(2 * HID, 3 * HID), np.arange(3 * HID, 4 * HID),
    ])
    bias = (b + b_conv)[perm]
    wxa = np.ascontiguousarray(
        np.concatenate([Wx[:, perm], bias[None, :]], axis=0), dtype=np.float32)
    whp = np.ascontiguousarray(Wh[:, perm], dtype=np.float32)
    wci_t = np.ascontiguousarray(np.tile(w_ci[None, :], (NPC, 1)), dtype=np.float32)
    wcf_t = np.ascontiguousarray(np.tile(w_cf[None, :], (NPC, 1)), dtype=np.float32)
    wco_t = np.ascontiguousarray(np.tile(w_co[None, :], (NPC, 1)), dtype=np.float32)

    # xs: [B,T,A,F] -> [T, N, F] with node n = b*A + a
    xs_full = np.transpose(agent_obs, (1, 0, 2, 3)).reshape(T, B * A, F_IN)
    in_maps = []
    for d in range(N_CORES):
        xs_d = xs_full[:, d * NPC:(d + 1) * NPC, :]          # [T, NPC, F]
        xsT = np.transpose(xs_d, (2, 0, 1)).reshape(F_IN, T * NPC)
        xs_aug = np.ascontiguousarray(
            np.concatenate([xsT, np.ones((1, T * NPC), np.float32)], axis=0),
            dtype=np.float32)
        in_maps.append({
            "xs": xs_aug, "wxa": wxa, "wh": whp,
            "wci": wci_t, "wcf": wcf_t, "wco": wco_t,
        })
    return in_maps


def kernel(agent_obs, hideout_obs, timestep_obs, Wx, Wh, b, b_conv,
           w_ci, w_cf, w_co, num_agents=None, _trace=False):
    agent_obs = np.asarray(agent_obs, dtype=np.float32)
    hideout_obs = np.asarray(hideout_obs, dtype=np.float32)
    timestep_obs = np.asarray(timestep_obs, dtype=np.float32)
    Wx = np.asarray(Wx, dtype=np.float32)
    Wh = np.asarray(Wh, dtype=np.float32)
    b = np.asarray(b, dtype=np.float32)
    b_conv = np.asarray(b_conv, dtype=np.float32)
    w_ci = np.asarray(w_ci, dtype=np.float32)
    w_cf = np.asarray(w_cf, dtype=np.float32)
    w_co = np.asarray(w_co, dtype=np.float32)
    assert agent_obs.shape == (B, T, A, F_IN)

    nc = _get_nc()
    in_maps = _prep_in_maps(agent_obs, Wx, Wh, b, b_conv, w_ci, w_cf, w_co)
    res = run_bass_kernel_spmd(nc, in_maps, core_ids=list(range(N_CORES)),
                               trace=_trace)
    _cache["last_results"] = res

    h_all = np.concatenate([res.results[d]["out"] for d in range(N_CORES)], axis=0)
    pooled = h_all.reshape(B, A, HID).mean(axis=1)
    return np.concatenate([pooled, hideout_obs, timestep_obs], axis=1).astype(np.float32)


# revision 33
# speedup vs baseline: 1.2046x; 1.2046x over previous
"""GC-LSTM (ChebConv K=1 + peephole LSTM + mean-pool) on 8 TRN2 NeuronCores.

Sharding: data-parallel over the node axis N = B*A = 1024 -> 128 nodes/core.
Gate weights are replicated; the T=128 recurrence runs fully on-chip per core
with no cross-device traffic. Mean-pool over agents + concat happen on host.

Per-core, per-step structure (gate column order permuted to [f, i, g, o]):
  - pre_g = [x_t;1] @ [Wx;bias]_g + h_{t-1} @ Wh_g accumulated in one PSUM
    bank per gate ([128 nodes, 384]); lhsT = transposed activations
    (xT 33x128 float32r, hT 128x128 bf16 chunks), rhs = weight column slices
    (N=384 keeps float32r/bf16 at 1 cycle/row).
  - f/i peephole terms (w_cf*c, w_ci*c) are accumulated into the same PSUM
    banks by the PE via [128,128] block-diagonal bf16 matmuls against the
    transposed cell state cT, so the sigmoids read PSUM directly.
  - elementwise runs in [nodes=128 partitions, HID=384 free]: sig(f), sig(i),
    tanh(g) on ACT (bf16 out), c_new = f*c + i*g on DVE (c stays fp32).
  - o-gate peephole uses c_new via DVE mul/add + ACT sigmoid; h = o * tanh(c).
  - c_new and h are transposed back to [HID, nodes] chunks on the PE
    (cT feeds the next step's diag matmuls, hT the next step's Wh matmuls).
  - a few dependent "warm" matmuls keep the PE HAM clock from idling down.
"""

import numpy as np
import ml_dtypes

import concourse.bass as bass
from concourse import bacc
import concourse.mybir as mybir
import concourse.tile as tile
from concourse.tile import add_dep_helper
from concourse.bass_utils import run_bass_kernel_spmd
from concourse.masks import make_identity

AF = mybir.ActivationFunctionType
F32 = mybir.dt.float32
F32R = mybir.dt.float32r
BF16 = mybir.dt.bfloat16

B, T, A, F_IN, HID = 64, 128, 16, 32, 384
H4 = 4 * HID
N_CORES = 8
NPC = (B * A) // N_CORES  # nodes per core = 128
KC = HID // 128           # h contraction chunks = 3
XR = F_IN + 1             # x rows + ones row for bias = 33
NXT = 4                   # xs DMA chunks


_cache = {}


def _build_nc():
    nc = bacc.Bacc("TRN2", target_bir_lowering=False, debug=False)

    d_xs = nc.dram_tensor("xs", [XR, T * NPC], F32R, kind="ExternalInput")
    d_wxa = nc.dram_tensor("wxa", [XR, H4], F32R, kind="ExternalInput")
    d_wh = nc.dram_tensor("wh", [HID, H4], BF16, kind="ExternalInput")
    d_dgf = nc.dram_tensor("dgf", [128, KC, 128], BF16, kind="ExternalInput")
    d_dgi = nc.dram_tensor("dgi", [128, KC, 128], BF16, kind="ExternalInput")
    d_wco = nc.dram_tensor("wco", [NPC, HID], F32, kind="ExternalInput")
    d_out = nc.dram_tensor("out", [NPC, HID], F32, kind="ExternalOutput")

    with tile.TileContext(nc) as tc:
        with (
            tc.tile_pool(name="consts", bufs=1) as cp,
            tc.tile_pool(name="work", bufs=2) as wp,
            tc.tile_pool(name="psum", bufs=2, space="PSUM") as pp,
        ):
            # ---------------- constants (DMA'd once) ----------------
            wxa = cp.tile([XR, H4], F32R, tag="wxa", name="wxa")
            nc.sync.dma_start(wxa, d_wxa.ap())
            wh = cp.tile([128, KC, H4], BF16, tag="wh", name="wh")
            for k in range(KC):
                nc.sync.dma_start(wh[:, k, :], d_wh.ap()[k * 128:(k + 1) * 128, :])
            dgf = cp.tile([128, KC, 128], BF16, tag="dgf", name="dgf")
            nc.sync.dma_start(dgf, d_dgf.ap())
            dgi = cp.tile([128, KC, 128], BF16, tag="dgi", name="dgi")
            nc.sync.dma_start(dgi, d_dgi.ap())
            wco = cp.tile([NPC, HID], F32, tag="wco", name="wco")
            nc.sync.dma_start(wco, d_wco.ap())
            xs_t = []
            xcols = T * NPC // NXT
            for q in range(NXT):
                xt = cp.tile([XR, xcols], F32R, tag=f"xs{q}", name=f"xs{q}")
                nc.sync.dma_start(xt, d_xs.ap()[:, q * xcols:(q + 1) * xcols])
                xs_t.append(xt)
            ident = cp.tile([128, 128], F32, tag="ident", name="ident")
            make_identity(nc, ident)
            ident_bf = cp.tile([128, 128], BF16, tag="identbf", name="ident_bf")
            nc.vector.tensor_copy(ident_bf, ident)

            # ---------------- state init ----------------
            c_prev = wp.tile([NPC, HID], F32, tag="c", bufs=2, name="c_init")
            nc.vector.memset(c_prev, 0.0)
            hT0 = wp.tile([128, KC, NPC], BF16, tag="hTa", bufs=2, name="hT_init")
            nc.gpsimd.memset(hT0.bitcast(mybir.dt.uint16), 0)
            hT = [hT0[:, k, :] for k in range(KC)]
            cT0 = wp.tile([128, KC, NPC], BF16, tag="cTa", bufs=2, name="cT_init")
            nc.gpsimd.memset(cT0.bitcast(mybir.dt.uint16), 0)
            cT = cT0

            # ---------------- time loop (fully unrolled) ----------------
            h = None
            for t in range(T):
                q, r = divmod(t, T // NXT)
                xT = xs_t[q][:, r * NPC:(r + 1) * NPC]

                # per-gate psum tiles: exact deps, each one bank
                pre_f = pp.tile([NPC, HID], F32, tag="pref", bufs=1, name=f"pref{t}")
                pre_i = pp.tile([NPC, HID], F32, tag="prei", bufs=1, name=f"prei{t}")
                pre_g = pp.tile([NPC, HID], F32, tag="preg", bufs=1, name=f"preg{t}")
                pre_o = pp.tile([NPC, HID], F32, tag="preo", bufs=1, name=f"preo{t}")
                # x-part + bias (start) — off the h critical path
                nc.tensor.matmul(pre_f, xT, wxa[:, 0:384], start=True, stop=False)
                nc.tensor.matmul(pre_i, xT, wxa[:, 384:768], start=True, stop=False)
                nc.tensor.matmul(pre_g, xT, wxa[:, 768:1152], start=True, stop=False)
                nc.tensor.matmul(pre_o, xT, wxa[:, 3 * HID:4 * HID],
                                 start=True, stop=False)
                # peephole diag accumulation (uses cT of t-1)
                diag_f, diag_i = [], []
                for k in range(KC):
                    ks = slice(128 * k, 128 * (k + 1))
                    mm = nc.tensor.matmul(pre_f[:, ks], cT[:, k, :], dgf[:, k, :],
                                          start=False, stop=False,
                                          skip_group_check=True)
                    diag_f.append(mm)
                    mm = nc.tensor.matmul(pre_i[:, ks], cT[:, k, :], dgi[:, k, :],
                                          start=False, stop=False,
                                          skip_group_check=True)
                    diag_i.append(mm)
                # h-part, k-outer so consecutive matmuls share lhsT (ldw reuse)
                for k in range(KC):
                    for pg, gs, deps in ((pre_f, slice(0, 384), diag_f),
                                         (pre_g, slice(768, 1152), None),
                                         (pre_i, slice(384, 768), diag_i),
                                         (pre_o, slice(1152, 1536), None)):
                        mm = nc.tensor.matmul(pg, hT[k], wh[:, k, gs],
                                              start=False, stop=(k == KC - 1))
                        if k == KC - 1 and deps:
                            for dmm in deps:
                                add_dep_helper(mm.ins, dmm.ins, sync=False,
                                               reason="diag before bank stop")

                # gates: columns [f, i, g, o]; f/i peephole already in psum
                f_ = wp.tile([NPC, HID], BF16, tag="f", bufs=3, name=f"f{t}")
                nc.scalar.activation(f_, pre_f, AF.Sigmoid)
                g = wp.tile([NPC, HID], BF16, tag="g", bufs=3, name=f"g{t}")
                nc.scalar.activation(g, pre_g, AF.Tanh)
                i_ = wp.tile([NPC, HID], BF16, tag="i", bufs=3, name=f"i{t}")
                nc.scalar.activation(i_, pre_i, AF.Sigmoid)

                warm1 = pp.tile([128, HID], F32, tag="warm", bufs=1,
                                name=f"warm1_{t}")
                nc.tensor.matmul(warm1, ident_bf, f_, start=True, stop=True)
                t1 = wp.tile([NPC, HID], F32, tag="t1", bufs=3, name=f"t1_{t}")
                nc.vector.tensor_mul(t1, f_, c_prev)
                t2 = wp.tile([NPC, HID], BF16, tag="t2", bufs=3, name=f"t2_{t}")
                nc.vector.tensor_mul(t2, i_, g)
                c_new = wp.tile([NPC, HID], F32, tag="c", bufs=2, name=f"c{t}")
                nc.vector.tensor_add(c_new, t1, t2)

                # transpose c_new (fp32) into psum for next step's f/i diag
                pTc = pp.tile([128, KC, NPC], F32, tag="pTc", bufs=1,
                              name=f"pTc{t}")
                for k in range(KC):
                    nc.tensor.transpose(pTc[:, k, :],
                                        c_new[:, 128 * k:128 * (k + 1)], ident)
                cTn = wp.tile([128, KC, NPC], BF16, tag="cTa", bufs=2,
                              name=f"cTa{t}")
                nc.vector.tensor_copy(cTn, pTc)

                warm2 = pp.tile([128, HID], F32, tag="warm", bufs=1,
                                name=f"warm2_{t}")
                nc.tensor.matmul(warm2, ident_bf, g, start=True, stop=True)
                to_ = wp.tile([NPC, HID], F32, tag="to", bufs=3, name=f"to{t}")
                nc.vector.tensor_mul(to_, c_new, wco)
                so_ = wp.tile([NPC, HID], F32, tag="so", bufs=3, name=f"so{t}")
                nc.vector.tensor_add(so_, to_, pre_o)
                o_ = wp.tile([NPC, HID], BF16, tag="o", bufs=3, name=f"o{t}")
                nc.scalar.activation(o_, so_, AF.Sigmoid)
                warm3 = pp.tile([128, HID], F32, tag="warm", bufs=1,
                                name=f"warm3_{t}")
                nc.tensor.matmul(warm3, ident_bf, o_, start=True, stop=True)
                th = wp.tile([NPC, HID], BF16, tag="th", bufs=3, name=f"th{t}")
                nc.scalar.activation(th, c_new, AF.Tanh)
                h = wp.tile([NPC, HID], BF16 if t != T - 1 else F32, tag="h",
                            bufs=2, name=f"h{t}")
                nc.vector.tensor_mul(h, o_, th)
                if t != T - 1:
                    pT = pp.tile([128, KC, NPC], BF16, tag="pT", bufs=2,
                                 name=f"pT{t}")
                    hTn = wp.tile([128, KC, NPC], BF16, tag="hTa", bufs=2,
                                  name=f"hTa{t}")
                    for k in range(KC):
                        nc.tensor.transpose(pT[:, k, :],
                                            h[:, 128 * k:128 * (k + 1)], ident_bf)
                    nc.vector.tensor_copy(hTn, pT)
                    hT = [hTn[:, k, :] for k in range(KC)]
                cT = cTn
                c_prev = c_new

            nc.sync.dma_start(d_out.ap(), h)

    nc.compile()
    return nc
